# revision 31
# baseline (speedup 1.0000x reference)
"""BERT-base encoder layer on 8 Trainium2 NeuronCores (Bass/Tile).

Sharding: data-parallel over batch. Full inputs [32, 512, 768] split into 8
shards of 4 batches (2048 tokens); every core runs the same NEFF on its shard
(SPMD, no collectives); host concatenates the outputs.

v3 design (vs v2 bf16 baseline): fp8 e4m3 DoubleRow matmuls on the PE
wherever the quantization error stays within the 2e-2 budget. A DoubleRow
matmul pairs two 128-row contraction slabs in one instruction (moving
[128,2,512] fp8 streamed 2 values/lane/cycle), halving PE time per
256-contraction vs bf16 (measured: 214ns per DR matmul vs 2x214 bf16).
- QKV and O-proj: pure fp8 DoubleRow (errors are diluted ~10x by the x
  residual before reaching the trunk).
- Attention A@V + softmax denominator: all fp8 DoubleRow. DR cannot target
  PSUM partition offset 64 (hw forces the full col-group), so the odd head
  of each pair uses a FULL-width 128-col stationary: rows 64-127 get its
  numerator/denominator at the correct lanes and rows 0-63 compute a
  garbage product that the even head's start=True 64-wide group then
  overwrites (the PE executes in order, so this is race-free).
- The key mask is multiplicative: folded into the V epilogue (per-partition
  cv*keep scalars) and into the denominator's fp8 stationary, so exp runs
  UNMASKED as one [128,1024] activation per head per key-chunk-pair
  (24/batch instead of 48 biased exps).
- FFN fractionally fp8: all 3 wi contraction pairs + 8 of 12 wout pairs in
  fp8 DoubleRow; the last 8 wout chunks stay bf16 (host pre-scales them by
  swout so the psum scale stays uniform). Measured rel err 1.83e-2 vs the
  2e-2 gate; a CPU quantization sim (sim_fp8.py) tracks hw to <1% relative.
- Trunk-critical Wso GEMM and QK^T score matmuls stay bf16 (Wso errors hit
  the residual stream 1:1; scores have only 64-deep contraction so fp8
  DoubleRow cannot apply anyway).
All fp8 descale constants fold into existing epilogue ops (copy consts, the
gelu input scale, the LN psum scale), so fp8 adds no vector/scalar work.
Scores are computed transposed (keys on partitions) exactly as in v2; exp
output IS the P^T pair-slab layout the A@V DoubleRow needs.

Superphase A = QKV + attention + O-proj (stores xa = x + attn out);
superphase B = SelfOutput GEMM + LN1 + FFN + LN2, software-pipelined
[wso(b+1), htrans(b), wout(b-1), wi(b)] so LN1 latency and the hT/ffT psum
drains hide under GEMM windows; wso(0)+LN1(0) runs in superphase A's tail
under the last exp window (with its LN writes on the scalar engine); the
final iteration runs wi(3) before wout(2) so wout(3) starts gelu-stall-free
(ffT pools sized for 2 batches to permit that reorder).
"""

import os
import numpy as np
import ml_dtypes

B, S, E, H, DK, FF = 32, 512, 768, 12, 64, 3072
NCORES = 8
BL = B // NCORES          # batches per core = 4
T = BL * S                # tokens per core = 2048
EPS = 1e-12

SX_DEF = 32.0             # x fp8 scale (absmax ~5.4 -> ~170 < 240)
SV = 8.0                  # v fp8 scale
SA = 8.0                  # attention-output fp8 scale
SH = 8.0                  # h fp8 scale

_CACHE = {}


def _bf(a):
    return np.ascontiguousarray(np.asarray(a, np.float32).astype(ml_dtypes.bfloat16))


def _f8(a):
    a = np.clip(np.asarray(a, np.float32), -240.0, 240.0)
    return np.ascontiguousarray(a.astype(ml_dtypes.float8_e4m3))


def _pack3(w):
    """(K, N) -> (K//256*128, 2, N): DoubleRow slab pairs along dim 1."""
    K, N = w.shape
    t = w.reshape(K // 256, 2, 128, N).transpose(0, 2, 1, 3)
    return np.ascontiguousarray(t.reshape(K // 256 * 128, 2, N))


def _build(key):
    import concourse.bass as bass
    import concourse.bacc as bacc
    import concourse.mybir as mybir
    import concourse.tile as tile
    from contextlib import ExitStack

    (flags, cq, ck, cv, cav, co, cg, cw, sh) = key
    (use_bq, use_bk, use_bv, use_bo, use_bso, use_bi, use_bout,
     use_g1, use_b1, use_g2, use_b2) = flags

    AF = mybir.ActivationFunctionType
    OP = mybir.AluOpType
    AX = mybir.AxisListType
    BF16 = mybir.dt.bfloat16
    F32 = mybir.dt.float32
    F8 = mybir.dt.float8e4
    DR = mybir.MatmulPerfMode.DoubleRow

    nc = bacc.Bacc("TRN2", target_bir_lowering=False)

    d_xt = nc.dram_tensor("xt", (E, T), BF16, kind="ExternalInput")
    d_x8 = nc.dram_tensor("x8", (E // 2, 2, T), F8, kind="ExternalInput")
    d_wq8 = nc.dram_tensor("wq8", (E // 2, 2, E), F8, kind="ExternalInput")
    d_wk8 = nc.dram_tensor("wk8", (E // 2, 2, E), F8, kind="ExternalInput")
    d_wv8 = nc.dram_tensor("wv8", (E // 2, 2, E), F8, kind="ExternalInput")
    d_wo8 = nc.dram_tensor("wo8", (E // 2, 2, E), F8, kind="ExternalInput")
    d_wso = nc.dram_tensor("wso", (E, E), BF16, kind="ExternalInput")
    # FFN weights: first WI_P8/WT_P8 contraction pairs ship as fp8
    # DoubleRow pairs; the remaining chunks ship bf16 (pre-scaled by the
    # same swi/swout so the psum scale stays uniform)
    d_wi8 = nc.dram_tensor("wi8", (384, 2, FF), F8, kind="ExternalInput")
    d_wib = nc.dram_tensor("wib", (1, FF), BF16, kind="ExternalInput")
    d_wt8 = nc.dram_tensor("wt8", (1024, 2, E), F8, kind="ExternalInput")
    d_wtb = nc.dram_tensor("wtb", (1024, E), BF16, kind="ExternalInput")
    # mask, as multiplicative keep columns: mkv[p, b*4+jc] = cv*keep (f32,
    # folded into the V epilogue) and mk8[p, b*4+jc, :] = keep broadcast 64
    # wide (fp8 stationary for the softmax-denominator matmuls)
    d_mkv = nc.dram_tensor("mkv", (128, BL * 4), F32, kind="ExternalInput")
    d_mk8 = nc.dram_tensor("mk8", (128, BL * 4, 128), F8, kind="ExternalInput")
    d_id = nc.dram_tensor("ident", (128, 128), BF16, kind="ExternalInput")
    d_ones = nc.dram_tensor("onesrow", (1, 512), BF16, kind="ExternalInput")
    # bias rows (host pre-scaled into fp8-psum units where applicable):
    # 0=bq/8*sx*swq, 1=bk*sx*swk, 2=bv*sx*swv, 3=bo*sa*swo, 4=bso,
    # 5=bout*swout, 6=bi
    d_brow = nc.dram_tensor("brow", (7, FF), BF16, kind="ExternalInput")
    d_bic = nc.dram_tensor("bicol", (128, FF // 128), F32, kind="ExternalInput")
    # gamma1 | gamma2 | beta1 | beta2 ([128, 768] each, partition-broadcast)
    # + a trailing always-zero [128, 512] scratch region
    d_gb = nc.dram_tensor("gb", (128, 4 * E + 512), F32, kind="ExternalInput")
    d_out = nc.dram_tensor("out", (T, E), F32, kind="ExternalOutput")

    KT_E = E // 128    # 6
    KP_E = E // 256    # 3 DoubleRow pairs over the embedding dim
    NT_B = S // 128    # 4
    FT = FF // 128     # 24
    FP = FF // 256     # 12 DoubleRow pairs over the FFN dim
    WI_P8 = 3          # wi contraction pairs in fp8 (of 3); rest bf16
    WT_P8 = 8          # wout contraction pairs in fp8 (of 12); rest bf16

    need_gb = use_g1 or use_b1 or use_g2 or use_b2
    need_brow = use_bq or use_bk or use_bv or use_bo or use_bso or use_bout

    with ExitStack() as ctx:
        tc = ctx.enter_context(tile.TileContext(nc))

        c_pool = ctx.enter_context(tc.tile_pool(name="consts", bufs=1))
        # Wso hoisted to outer scope so its DMA overlaps superphase A
        wso_pool = ctx.enter_context(tc.tile_pool(name="wso", bufs=KT_E))
        # xa = x + attention output (feature-major), crosses the A->B boundary
        xa_pool = ctx.enter_context(tc.tile_pool(name="xa", bufs=BL * KT_E))
        # LN machinery is shared between the phases: wso(0)+LN1(0) runs in
        # superphase A's tail (PE slack under the last exp window)
        h_pool = ctx.enter_context(tc.tile_pool(name="h", bufs=3 * NT_B))
        sq_pool = ctx.enter_context(tc.tile_pool(name="sq", bufs=2))
        rs_pool = ctx.enter_context(tc.tile_pool(name="rsd", bufs=2))
        t_pool = ctx.enter_context(tc.tile_pool(name="sb_s", bufs=12))

        XA = {}    # (b, kt) -> [128, S] bf16 tile

        def layernorm(chunks, h_dst, gcol, use_g, use_bb, resid=None,
                      pscale=1.0, scalar_writes=False):
            """chunks: [(psum_ap, col0, n)]; h_dst: [128, E] out.
            resid: parallel list of sbuf APs added to psum*pscale first.

            Stages the psum chunks into SBUF immediately (frees the PSUM
            banks for the next GEMM group); eps=1e-12 dropped (var >> eps).
            References ones/gb/zeros, which are loaded before first use."""
            rtile = rs_pool.tile([128, E], F32, name="rt", tag="rsd")
            s1 = t_pool.tile([128, 1], F32, name="s1", tag="s1")
            s1b = t_pool.tile([128, 1], F32, name="s1b", tag="s1b")
            for i, ((ps, c0, n), acc) in enumerate(zip(chunks, (s1, s1b))):
                rx = resid[i] if resid is not None else zeros[:, :n]
                nc.vector.scalar_tensor_tensor(
                    rtile[:, c0:c0 + n], ps, pscale, rx,
                    op0=OP.mult, op1=OP.add)
                nc.vector.reduce_sum(acc[:, :], rtile[:, c0:c0 + n],
                                     axis=AX.X)
            srcs = [(rtile[:, c0:c0 + n], c0, n) for (_, c0, n) in chunks]
            mu_n = t_pool.tile([128, 1], F32, name="mun", tag="mun")
            nc.vector.tensor_scalar(           # mu_n = -(s1 + s1b)/E
                mu_n[:, :], s1[:, :], s1b[:, :], -1.0 / E,
                op0=OP.add, op1=OP.mult)
            ss = t_pool.tile([128, 1], F32, name="ssa", tag="ssa", bufs=34)
            ssb = t_pool.tile([128, 1], F32, name="ssb", tag="ssb", bufs=34)
            for (src, c0, n), acc in zip(srcs, (ss, ssb)):
                sq = sq_pool.tile([128, 512], BF16, name="sqt", tag="sq")
                nc.scalar.activation(sq[:, :n], src, AF.Square,
                                     accum_out=acc[:, :])
            v1 = t_pool.tile([128, 1], F32, name="v1", tag="v1")
            nc.vector.tensor_scalar(           # (ss+ssb)/E
                v1[:, :], ss[:, :], ssb[:, :], 1.0 / E,
                op0=OP.add, op1=OP.mult)
            musq = t_pool.tile([128, 1], F32, name="musq", tag="musq")
            nc.vector.scalar_tensor_tensor(    # mu^2
                musq[:, :], mu_n[:, :], 1.0, mu_n[:, :],
                op0=OP.mult, op1=OP.mult)
            var = t_pool.tile([128, 1], F32, name="var", tag="var")
            nc.vector.scalar_tensor_tensor(    # var = v1 - mu^2
                var[:, :], musq[:, :], -1.0, v1[:, :],
                op0=OP.mult, op1=OP.add)
            sd = t_pool.tile([128, 1], F32, name="sd", tag="sd")
            nc.scalar.sqrt(sd[:, :], var[:, :])
            rstd = t_pool.tile([128, 1], F32, name="rstd", tag="rstd")
            nc.vector.reciprocal(rstd[:, :], sd[:, :])
            if scalar_writes:
                # (x + mu_n)*rstd == x*rstd + mu_n*rstd -> scalar ACT
                mnr = t_pool.tile([128, 1], F32, name="mnr", tag="mnr")
                nc.vector.scalar_tensor_tensor(
                    mnr[:, :], mu_n[:, :], 1.0, rstd[:, :],
                    op0=OP.mult, op1=OP.mult)
                for (src, c0, n) in srcs:
                    nc.scalar.activation(
                        h_dst[:, c0:c0 + n], src, AF.Identity,
                        bias=mnr[:, :], scale=rstd[:, :])
            else:
                for (src, c0, n) in srcs:      # (x - mu) * rstd
                    nc.vector.tensor_scalar(
                        h_dst[:, c0:c0 + n], src, mu_n[:, :], rstd[:, :],
                        op0=OP.add, op1=OP.mult)
            if use_g:
                nc.vector.scalar_tensor_tensor(
                    h_dst[:, :], h_dst[:, :], 1.0,
                    gb[:, gcol * E:(gcol + 1) * E],
                    op0=OP.mult, op1=OP.mult)
            if use_bb:
                nc.vector.scalar_tensor_tensor(
                    h_dst[:, :], h_dst[:, :], 1.0,
                    gb[:, (gcol + 2) * E:(gcol + 3) * E],
                    op0=OP.mult, op1=OP.add)

        def wso_tt(b, tt, pmm, scalar_writes=False):
            """One token-tile of the SelfOutput GEMM + LN1 -> h tile (bf16)."""
            ch = []
            for ec, n in ((0, 512), (512, 256)):
                ps = pmm.tile([128, 512], F32, name="sops", tag="mm")
                for k in range(KT_E):
                    nc.tensor.matmul(
                        ps[:, :n], XA[(b, k)][:, tt * 128:(tt + 1) * 128],
                        WSO[k][:, ec:ec + n],
                        start=(k == 0), stop=(k == KT_E - 1 and not use_bso))
                if use_bso:
                    nc.tensor.matmul(
                        ps[:, :n], ones[0:1, 0:128],
                        brow[4:5, ec:ec + n], start=False, stop=True)
                ch.append((ps[:, :n], ec, n))
            ht = h_pool.tile([128, E], BF16, name="hht", tag="h")
            layernorm(ch, ht, 0, use_g1, use_b1, scalar_writes=scalar_writes)
            return ht

        # ============== superphase A: QKV, attention, O-proj ==============
        with ExitStack() as sa:
            w8_pool = sa.enter_context(tc.tile_pool(name="w8", bufs=KP_E))
            xt_pool = sa.enter_context(tc.tile_pool(name="xt",
                                                    bufs=2 * KT_E))
            x8_pool = sa.enter_context(tc.tile_pool(name="x8",
                                                    bufs=2 * KP_E + 1))
            qt_pool = sa.enter_context(tc.tile_pool(name="qt", bufs=KT_E + 1))
            kt_pool = sa.enter_context(tc.tile_pool(name="kt", bufs=KT_E + 1))
            v_pool = sa.enter_context(tc.tile_pool(name="v", bufs=4))
            pexp_pool = sa.enter_context(tc.tile_pool(name="pexp", bufs=24))
            rsb_pool = sa.enter_context(tc.tile_pool(name="rsb", bufs=2))
            att_pool = sa.enter_context(tc.tile_pool(name="att", bufs=7))

            p_mm = sa.enter_context(tc.tile_pool(name="p_mm", bufs=2,
                                                 space="PSUM"))
            p_sc = sa.enter_context(tc.tile_pool(name="p_sc", bufs=2,
                                                 space="PSUM"))
            p_acc = sa.enter_context(tc.tile_pool(name="p_acc", bufs=2,
                                                  space="PSUM"))

            def load_x8(b, split=False):
                x8s = []
                engs = ((nc.gpsimd, nc.sync, nc.scalar) if split
                        else (nc.gpsimd, nc.gpsimd, nc.gpsimd))
                for kp in range(KP_E):
                    t = x8_pool.tile([128, 2, S], F8, name="x8t", tag="x8")
                    engs[kp].dma_start(
                        t[:, :, :],
                        d_x8[kp * 128:(kp + 1) * 128, :, b * S:(b + 1) * S])
                    x8s.append(t)
                return x8s

            def load_xt(b, split=False):
                xts = []
                for k in range(KT_E):
                    t = xt_pool.tile([128, S], BF16, name="xtt", tag="xt")
                    eng = nc.sync if (split and k >= 4) else nc.gpsimd
                    eng.dma_start(
                        t[:, :], d_xt[k * 128:(k + 1) * 128, b * S:(b + 1) * S])
                    xts.append(t)
                return xts

            def load_x(b):
                return load_xt(b), load_x8(b)

            # startup: the first qk matmul only needs WQ8[0]+X8[0]; lead
            # each DMA queue with one of those, everything else behind
            def _load8(dram, kp, eng, name, width=E):
                t = w8_pool.tile([128, 2, width], F8, name=name, tag=name)
                eng.dma_start(t[:, :, :], dram[kp * 128:(kp + 1) * 128, :, :])
                return t

            WQ8 = [_load8(d_wq8, kp, (nc.sync, nc.scalar, nc.gpsimd)[kp],
                          "wqt") for kp in range(KP_E)]
            X8_cur = load_x8(0, split=True)
            WK8 = [_load8(d_wk8, kp, (nc.scalar, nc.sync, nc.gpsimd)[kp],
                          "wkt") for kp in range(KP_E)]
            XT_cur = load_xt(0, split=True)
            mkv = c_pool.tile_from(d_mkv[:, :], name="mkv")
            mk8 = c_pool.tile([128, BL * 4, 128], F8, name="mk8")
            nc.scalar.dma_start(mk8[:, :, :], d_mk8[:, :, :])
            # prewarm the EXP act-table while the PE waits on weight DMAs,
            # taking the 1.3us table load off the first score's critical path
            warm = t_pool.tile([128, 1], F32, name="warm", tag="warm")
            nc.scalar.activation(warm[:, :], mkv[:, 0:1], AF.Exp)
            WV8 = [_load8(d_wv8, kp, (nc.sync if kp % 2 == 0
                                      else nc.gpsimd), "wvt")
                   for kp in range(KP_E)]
            WO8 = [_load8(d_wo8, kp, (nc.scalar if kp % 2 == 0
                                      else nc.sync), "wot")
                   for kp in range(KP_E)]
            WSO = [wso_pool.tile([128, E], BF16, name="wsot", tag="wsot")
                   for _ in range(KT_E)]
            for k in range(KT_E):
                nc.sync.dma_start(WSO[k][:, :],
                                  d_wso[k * 128:(k + 1) * 128, :])
            ident = c_pool.tile_from(d_id[:, :], name="ident")
            ones = c_pool.tile_from(d_ones[:, :], name="ones")
            # trailing columns of d_gb are always zero-filled by the host
            zeros = c_pool.tile_from(d_gb[:, 4 * E:4 * E + 512], name="zeros")
            brow = (c_pool.tile_from(d_brow[:, :], name="brow")
                    if need_brow else None)
            gb = c_pool.tile_from(d_gb[:, :], name="gb") if need_gb else None

            ATT_prev, XT_prev = None, None
            H0 = []    # h(0) tiles produced in this phase's tail

            for b in range(BL):
                XT, X8 = XT_cur, X8_cur
                if b + 1 < BL:
                    XT_next, X8_next = load_x(b + 1)

                QT, KTt = [None] * KT_E, [None] * KT_E
                V8 = [None] * 2
                PEXP = {}
                ATT8 = [None] * KP_E

                def qk(et):
                    for W8, dstl, pool, ub, brx, tg, cst in (
                            (WQ8, QT, qt_pool, use_bq, 0, "qt", cq),
                            (WK8, KTt, kt_pool, use_bk, 1, "kt", ck)):
                        ps = p_mm.tile([128, S], F32, name="qkps", tag="mm")
                        for kp in range(KP_E):
                            nc.tensor.matmul(
                                ps[:, :],
                                W8[kp][:, :, et * 128:(et + 1) * 128],
                                X8[kp][:, :, :], perf_mode=DR,
                                start=(kp == 0),
                                stop=(kp == KP_E - 1 and not ub))
                        if ub:
                            nc.tensor.matmul(
                                ps[:, :],
                                brow[brx:brx + 1, et * 128:(et + 1) * 128],
                                ones[0:1, 0:S], start=False, stop=True)
                        dstl[et] = pool.tile([128, S], BF16, name="qkt",
                                             tag=tg)
                        nc.vector.tensor_scalar_mul(dstl[et][:, :], ps[:, :],
                                                    cst)

                def sc(hp, it0):
                    # scores^T for head pair hp, key chunks it0, it0+1:
                    # psum[k-chunk, q] = (KT[h] chunk).T @ QT[h]; exp is
                    # UNMASKED (mask lives in the V rows + denominator)
                    p = it0 // 2
                    for hh in range(2):
                        o = hh * 64
                        ps = p_sc.tile([128, 2 * S], F32, name="scps",
                                       tag="sc")
                        for it in (it0, it0 + 1):
                            nc.tensor.matmul(
                                ps[:, (it - it0) * S:(it - it0 + 1) * S],
                                KTt[hp][o:o + 64, it * 128:(it + 1) * 128],
                                QT[hp][o:o + 64, :], start=True, stop=True)
                        hd = 2 * hp + hh
                        if PEXP.get((hd, p)) is None:
                            PEXP[(hd, p)] = pexp_pool.tile(
                                [128, 2, S], F8, name="pexp", tag="pe")
                        nc.scalar.activation(
                            PEXP[(hd, p)][:, :, :], ps[:, :], AF.Exp)

                def vproj_chunk(tt, ec, n):
                    if V8[tt // 2] is None:
                        V8[tt // 2] = v_pool.tile([128, 2, E], F8, name="vt",
                                                  tag="v")
                    ps = p_mm.tile([128, 512], F32, name="vps", tag="mm")
                    for kp in range(KP_E):
                        nc.tensor.matmul(
                            ps[:, :n],
                            X8[kp][:, :, tt * 128:(tt + 1) * 128],
                            WV8[kp][:, :, ec:ec + n], perf_mode=DR,
                            start=(kp == 0),
                            stop=(kp == KP_E - 1 and not use_bv))
                    if use_bv:
                        nc.tensor.matmul(
                            ps[:, :n], ones[0:1, 0:128],
                            brow[2:3, ec:ec + n], start=False, stop=True)
                    nc.vector.tensor_scalar_mul(
                        V8[tt // 2][:, tt % 2, ec:ec + n], ps[:, :n],
                        mkv[:, b * 4 + tt:b * 4 + tt + 1])

                def vproj(tt):
                    vproj_chunk(tt, 0, 512)
                    vproj_chunk(tt, 512, 256)

                def oproj(et):
                    ps = p_mm.tile([128, S], F32, name="ops", tag="mm")
                    for p in range(KP_E):
                        nc.tensor.matmul(
                            ps[:, :], WO8[p][:, :, et * 128:(et + 1) * 128],
                            ATT_prev[p][:, :, :], perf_mode=DR,
                            start=(p == 0),
                            stop=(p == KP_E - 1 and not use_bo))
                    if use_bo:
                        nc.tensor.matmul(
                            ps[:, :], brow[3:4, et * 128:(et + 1) * 128],
                            ones[0:1, 0:S], start=False, stop=True)
                    xat = xa_pool.tile([128, S], BF16, name="xat", tag="xa")
                    nc.vector.scalar_tensor_tensor(
                        xat[:, :], ps[:, :], co, XT_prev[et][:, :],
                        op0=OP.mult, op1=OP.add)
                    XA[(b - 1, et)] = xat

                def av(hp):
                    # DoubleRow cannot target psum partition 64 (col-group
                    # clash), but a FULL-width (128-col) stationary lands the
                    # odd head's numerator on rows 64-127 legally -- rows
                    # 0-63 compute a garbage product against V_even that the
                    # even head's start=True 64-wide group then overwrites
                    # (PE executes in order, so no race).
                    aps = p_acc.tile([128, S], F32, name="avps", tag="acc")
                    sps = p_acc.tile([128, S], F32, name="smps", tag="acc")
                    mks = mk8[:, b * 4:b * 4 + 4, :]
                    for p in range(2):   # odd head: rows 64-127 (+garbage)
                        nc.tensor.matmul(
                            aps[:, :],
                            V8[p][:, :, hp * 128:(hp + 1) * 128],
                            PEXP[(2 * hp + 1, p)][:, :, :],
                            perf_mode=DR, start=(p == 0), stop=(p == 1))
                        nc.tensor.matmul(
                            sps[:, :], mks[:, 2 * p:2 * p + 2, :],
                            PEXP[(2 * hp + 1, p)][:, :, :],
                            perf_mode=DR, start=(p == 0), stop=(p == 1))
                    for p in range(2):   # even head: resets rows 0-63
                        nc.tensor.matmul(
                            aps[0:64, :],
                            V8[p][:, :, hp * 128:hp * 128 + 64],
                            PEXP[(2 * hp, p)][:, :, :],
                            perf_mode=DR, start=(p == 0), stop=(p == 1),
                            tile_position=(0, 0), skip_group_check=True)
                        nc.tensor.matmul(
                            sps[0:64, :], mks[:, 2 * p:2 * p + 2, 0:64],
                            PEXP[(2 * hp, p)][:, :, :],
                            perf_mode=DR, start=(p == 0), stop=(p == 1),
                            tile_position=(0, 0), skip_group_check=True)
                    rsb = rsb_pool.tile([128, S], F32, name="rsb", tag="rsb")
                    # ~18-bit reciprocal, ~5x faster than nc.vector.reciprocal;
                    # sums are in [~1, 600] so no denorm/inf edge cases
                    nc.vector.reciprocal_approx_fast(rsb[:, :], sps[:, :])
                    if ATT8[hp // 2] is None:
                        ATT8[hp // 2] = att_pool.tile([128, 2, S], F8,
                                                      name="attt", tag="att")
                    nc.vector.scalar_tensor_tensor(
                        ATT8[hp // 2][:, hp % 2, :], aps[:, :], cav,
                        rsb[:, :], op0=OP.mult, op1=OP.mult)

                nop = lambda: None
                O = [(lambda et=et: oproj(et)) if b > 0 else nop
                     for et in range(KT_E)]
                # batch 3 also runs wso(0)+LN1(0) under its exp-window slack
                W0 = ([(lambda tt=tt: H0.append(wso_tt(0, tt, p_mm)))
                       for tt in range(NT_B)] if b == BL - 1 else [nop] * 4)
                # interleaved emission: exp window overlaps V/O-proj GEMMs
                sched = [
                    lambda: qk(0), lambda: qk(1),
                    lambda: sc(0, 0), lambda: qk(2),
                    lambda: sc(0, 2), lambda: qk(3),
                    lambda: sc(1, 0), lambda: qk(4),
                    lambda: sc(1, 2), lambda: qk(5),
                    lambda: sc(2, 0), lambda: vproj(0),
                    lambda: sc(2, 2), lambda: vproj(1),
                    lambda: sc(3, 0), lambda: vproj(2),
                    lambda: sc(3, 2), lambda: vproj(3),
                    lambda: sc(4, 0), lambda: av(0),
                    lambda: sc(4, 2), O[0],
                    lambda: sc(5, 0), O[1],
                    lambda: sc(5, 2), O[2],
                    lambda: av(1), O[3],
                    W0[0], lambda: av(2),
                    O[4], W0[1],
                    lambda: av(3), O[5],
                    W0[2], W0[3],
                    lambda: av(4), lambda: av(5),
                ]
                for seg in sched:
                    seg()

                ATT_prev = ATT8
                XT_prev = XT
                if b + 1 < BL:
                    XT_cur, X8_cur = XT_next, X8_next

            # O-projection for the last batch
            for et in range(KT_E):
                ps = p_mm.tile([128, S], F32, name="ops", tag="mm")
                for p in range(KP_E):
                    nc.tensor.matmul(
                        ps[:, :], WO8[p][:, :, et * 128:(et + 1) * 128],
                        ATT_prev[p][:, :, :], perf_mode=DR,
                        start=(p == 0), stop=(p == KP_E - 1 and not use_bo))
                if use_bo:
                    nc.tensor.matmul(
                        ps[:, :], brow[3:4, et * 128:(et + 1) * 128],
                        ones[0:1, 0:S], start=False, stop=True)
                xat = xa_pool.tile([128, S], BF16, name="xat", tag="xa")
                nc.vector.scalar_tensor_tensor(
                    xat[:, :], ps[:, :], co, XT_prev[et][:, :],
                    op0=OP.mult, op1=OP.add)
                XA[(BL - 1, et)] = xat

        # ========= superphase B: SelfOutput GEMM + LN1, FFN, LN2 =========
        with ExitStack() as sb:
            wi_pool = sb.enter_context(tc.tile_pool(name="wi", bufs=WI_P8))
            wib_pool = (sb.enter_context(tc.tile_pool(
                name="wib", bufs=KT_E - 2 * WI_P8))
                if KT_E > 2 * WI_P8 else None)
            wout_pool = sb.enter_context(tc.tile_pool(name="wout",
                                                      bufs=WT_P8))
            wtb_pool = sb.enter_context(tc.tile_pool(name="wtb",
                                                     bufs=FT - 2 * WT_P8))
            b_pool = sb.enter_context(tc.tile_pool(name="b_consts", bufs=1))
            ht_pool = sb.enter_context(tc.tile_pool(name="ht",
                                                    bufs=2 * WI_P8 + 1))
            htb_pool = (sb.enter_context(tc.tile_pool(
                name="htb", bufs=2 * (KT_E - 2 * WI_P8) + 1))
                if KT_E > 2 * WI_P8 else None)
            fft_pool = sb.enter_context(tc.tile_pool(name="fft",
                                                     bufs=2 * WT_P8 + 2))
            fftb_pool = sb.enter_context(tc.tile_pool(
                name="fftb", bufs=2 * (FT - 2 * WT_P8) + 2))
            out_pool = sb.enter_context(tc.tile_pool(name="outp", bufs=2))

            p_mm = sb.enter_context(tc.tile_pool(name="pb_mm", bufs=5,
                                                 space="PSUM"))
            p_tr = sb.enter_context(tc.tile_pool(name="pb_tr", bufs=3,
                                                 space="PSUM"))

            ENGS = (nc.sync, nc.gpsimd, nc.scalar)

            def _loadb(pool, dram, kp, name, qi):
                w = dram.shape[2]
                t = pool.tile([128, 2, w], F8, name=name, tag=name)
                ENGS[qi % 3].dma_start(t[:, :, :],
                                       dram[kp * 128:(kp + 1) * 128, :, :])
                return t

            WI8 = [_loadb(wi_pool, d_wi8, kp, "wi8t", kp)
                   for kp in range(WI_P8)]
            WIB = [wib_pool.tile([128, FF], BF16, name="wibt", tag="wibt")
                   for _ in range(KT_E - 2 * WI_P8)]
            for c in range(KT_E - 2 * WI_P8):
                ENGS[(WI_P8 + c) % 3].dma_start(
                    WIB[c][:, :], d_wib[c * 128:(c + 1) * 128, :])
            WT8 = [_loadb(wout_pool, d_wt8, fp, "wt8t", fp)
                   for fp in range(WT_P8)]
            WTB = [wtb_pool.tile([128, E], BF16, name="wtbt", tag="wtbt")
                   for _ in range(FT - 2 * WT_P8)]
            for c in range(FT - 2 * WT_P8):
                ENGS[(WT_P8 + c) % 3].dma_start(
                    WTB[c][:, :], d_wtb[c * 128:(c + 1) * 128, :])
            bic = b_pool.tile_from(d_bic[:, :], name="bic") if use_bi else None

            def emit_htrans(hh_t, sc_copies=False):
                hT = [ht_pool.tile([128, 2, S], F8, name="htt", tag="ht")
                      for _ in range(WI_P8)]
                hTb = [htb_pool.tile([128, S], BF16, name="htbt", tag="htb")
                       for _ in range(KT_E - 2 * WI_P8)]
                for tt in range(NT_B):
                    tps = [p_tr.tile([128, 512], BF16, name="htp", tag="tr")
                           for _ in range(2)]
                    for et in range(KT_E):
                        sl = tps[et // 4][:, (et % 4) * 128:(et % 4 + 1) * 128]
                        nc.tensor.transpose(
                            sl, hh_t[tt][:, et * 128:(et + 1) * 128],
                            ident[:, :])
                    for et in range(KT_E):
                        sl = tps[et // 4][:, (et % 4) * 128:(et % 4 + 1) * 128]
                        if et < 2 * WI_P8:
                            dst = hT[et // 2][:, et % 2,
                                              tt * 128:(tt + 1) * 128]
                        else:
                            dst = hTb[et - 2 * WI_P8][:,
                                                      tt * 128:(tt + 1) * 128]
                        if sc_copies:
                            nc.scalar.activation(dst, sl, AF.Copy,
                                                 scale=float(sh))
                        else:
                            nc.vector.tensor_scalar_mul(dst, sl, sh)
                return hT, hTb

            def emit_wi(hTp):
                hT, hTb = hTp
                nb = KT_E - 2 * WI_P8
                ffT = [None] * FP
                ffTb = [None] * (FT - 2 * WT_P8)
                for ft in range(FT):
                    ps = p_mm.tile([128, 512], F32, name="fips", tag="mm")
                    for kp in range(WI_P8):
                        nc.tensor.matmul(
                            ps[:, :], WI8[kp][:, :, ft * 128:(ft + 1) * 128],
                            hT[kp][:, :, :], perf_mode=DR,
                            start=(kp == 0),
                            stop=(kp == WI_P8 - 1 and nb == 0))
                    for c in range(nb):
                        nc.tensor.matmul(
                            ps[:, :], WIB[c][:, ft * 128:(ft + 1) * 128],
                            hTb[c][:, :],
                            start=False, stop=(c == nb - 1))
                    if ft < 2 * WT_P8:
                        if ffT[ft // 2] is None:
                            ffT[ft // 2] = fft_pool.tile(
                                [128, 2, 512], F8, name="fftt", tag="fft")
                        dst = ffT[ft // 2][:, ft % 2, :]
                    else:
                        ffTb[ft - 2 * WT_P8] = fftb_pool.tile(
                            [128, 512], BF16, name="fftbt", tag="fftb")
                        dst = ffTb[ft - 2 * WT_P8][:, :]
                    if use_bi:
                        nc.scalar.activation(dst, ps[:, :], AF.Gelu,
                                             bias=bic[:, ft:ft + 1], scale=cg)
                    else:
                        nc.scalar.activation(dst, ps[:, :], AF.Gelu, scale=cg)
                return ffT, ffTb

            def emit_wout(b, ffp, hh_t):
                ffT, ffTb = ffp
                t0 = b * S
                for tt in range(NT_B):
                    ch = []
                    nb = FT - 2 * WT_P8
                    for ec, n in ((0, 512), (512, 256)):
                        ps = p_mm.tile([128, 512], F32, name="wops", tag="mm")
                        for fp in range(WT_P8):
                            nc.tensor.matmul(
                                ps[:, :n],
                                ffT[fp][:, :, tt * 128:(tt + 1) * 128],
                                WT8[fp][:, :, ec:ec + n], perf_mode=DR,
                                start=(fp == 0), stop=False)
                        for c in range(nb):
                            nc.tensor.matmul(
                                ps[:, :n],
                                ffTb[c][:, tt * 128:(tt + 1) * 128],
                                WTB[c][:, ec:ec + n],
                                start=False,
                                stop=(c == nb - 1 and not use_bout))
                        if use_bout:
                            nc.tensor.matmul(
                                ps[:, :n], ones[0:1, 0:128],
                                brow[5:6, ec:ec + n], start=False, stop=True)
                        ch.append((ps[:, :n], ec, n))
                    otile = out_pool.tile([128, E], F32, name="ot", tag="outp")
                    resid = [hh_t[tt][:, ec:ec + n] for (_, ec, n) in ch]
                    layernorm(ch, otile, 1, use_g2, use_b2, resid=resid,
                              pscale=cw)
                    # sync queue keeps the slow gpsimd drain off the tail
                    nc.sync.dma_start(
                        d_out[t0 + tt * 128:t0 + (tt + 1) * 128, :],
                        otile[:, :])

            # software pipeline: h(0) computed in superphase A's tail (H0).
            # Per iteration: ht(b), wso(b+1), wout(b-1), wi(b) -- the LN1 and
            # LN2 sqrt activations cluster together between gelu runs (the
            # SQRT and GELU act-tables evict each other; interleaving them
            # per-token-tile cost ~19 table reloads of 1.3us each), and
            # wout(b-1) starts a full iteration after its gelus finished.
            h_ = [None] * BL
            hT_ = [None] * BL
            ff_ = [None] * BL
            h_[0] = H0
            h_[1] = [wso_tt(1, tt, p_mm, scalar_writes=True)
                     for tt in range(NT_B)]
            hT_[0] = emit_htrans(h_[0], sc_copies=True)
            # wso(2)'s GEMM window covers hT(0)'s scalar copy drain so wi(0)
            # starts copy-stall-free
            h_[2] = [wso_tt(2, tt, p_mm) for tt in range(NT_B)]
            ff_[0] = emit_wi(hT_[0])
            hT_[1] = emit_htrans(h_[1])
            emit_wout(0, ff_[0], h_[0])
            ff_[1] = emit_wi(hT_[1])
            h_[3] = [wso_tt(3, tt, p_mm) for tt in range(NT_B)]
            hT_[2] = emit_htrans(h_[2])
            emit_wout(1, ff_[1], h_[1])
            ff_[2] = emit_wi(hT_[2])
            # last iteration: wi(3) before wout(2) so its gelus finish under
            # wout(2)'s matmul window and wout(3) starts without a stall
            hT_[3] = emit_htrans(h_[3])
            ff_[3] = emit_wi(hT_[3])
            emit_wout(2, ff_[2], h_[2])
            emit_wout(3, ff_[3], h_[3])

    nc.compile()
    return nc


def _get_program(key):
    if key not in _CACHE:
        _CACHE[key] = _build(key)
    return _CACHE[key]


def kernel(x, mask, Wq, bq, Wk, bk, Wv, bv, Wo, bo,
           Wso, bso, gso, beso, Wi, bi, Wout, bout, gout, beout):
    from concourse.bass_utils import run_bass_kernel_spmd

    x = np.asarray(x, np.float32)
    mask = np.asarray(mask)
    sc = 1.0 / float(np.sqrt(np.float32(DK)))

    wq_t = np.asarray(Wq, np.float32) * sc
    amax = lambda a: max(float(np.abs(np.asarray(a, np.float32)).max()), 1e-30)
    sx = min(SX_DEF, 224.0 / amax(x))
    swq, swk = 224.0 / amax(wq_t), 224.0 / amax(Wk)
    swv, swo = 224.0 / amax(Wv), 224.0 / amax(Wo)
    swi, swout = 224.0 / amax(Wi), 224.0 / amax(Wout)

    cq = 1.0 / (sx * swq)
    ck = 1.0 / (sx * swk)
    cv = SV / (sx * swv)
    cav = SA / SV
    co = 1.0 / (SA * swo)
    cg = 1.0 / (SH * swi)
    cw = 1.0 / swout

    z = lambda a: not np.any(np.asarray(a))
    one = lambda a: bool(np.all(np.asarray(a) == 1.0))
    flags = (not z(bq), not z(bk), not z(bv), not z(bo), not z(bso),
             not z(bi), not z(bout),
             not one(gso), not z(beso), not one(gout), not z(beout))
    key = (flags, cq, ck, cv, cav, co, cg, cw, SH)
    nc = _get_program(key)

    wq8 = _pack3(_f8(wq_t * swq))
    wk8 = _pack3(_f8(np.asarray(Wk, np.float32) * swk))
    wv8 = _pack3(_f8(np.asarray(Wv, np.float32) * swv))
    wo8 = _pack3(_f8(np.asarray(Wo, np.float32) * swo))
    wi_s = np.asarray(Wi, np.float32) * swi
    wt_s = np.asarray(Wout, np.float32) * swout
    wi8 = _pack3(_f8(wi_s))
    wib = _bf(np.zeros((1, FF), np.float32))
    wt8 = _pack3(_f8(wt_s[:2048]))
    wtb = _bf(wt_s[2048:])
    wso_b = _bf(Wso)
    identb = _bf(np.eye(128))
    onesr = _bf(np.ones((1, 512)))

    brow = np.zeros((7, FF), np.float32)
    brow[0, :E] = np.asarray(bq, np.float32) * sc * (sx * swq)
    brow[1, :E] = np.asarray(bk, np.float32) * (sx * swk)
    brow[2, :E] = np.asarray(bv, np.float32) * (sx * swv)
    brow[3, :E] = np.asarray(bo, np.float32) * (SA * swo)
    brow[4, :E] = bso
    brow[5, :E] = np.asarray(bout, np.float32) * swout
    brow[6, :] = bi
    brow = _bf(brow)
    bicol = np.asarray(bi, np.float32).reshape(FF // 128, 128).T.copy()
    gbt = np.zeros((128, 4 * E + 512), np.float32)
    for i, g in enumerate((gso, gout, beso, beout)):   # g1|g2|b1|b2
        gbt[:, i * E:(i + 1) * E] = np.broadcast_to(
            np.asarray(g, np.float32).reshape(1, E), (128, E))

    in_maps = []
    for c in range(NCORES):
        xs = x[c * BL:(c + 1) * BL].reshape(T, E)
        xsT = np.ascontiguousarray(xs.T)
        xt = _bf(xsT)
        x8 = _f8(xsT * sx).reshape(E // 256, 2, 128, T)
        x8 = np.ascontiguousarray(
            x8.transpose(0, 2, 1, 3).reshape(E // 2, 2, T))
        ms = np.asarray(mask[c * BL:(c + 1) * BL]).reshape(BL, S)
        # keep[p, b*4 + jc] = 0/1 for key token jc*128+p of batch b
        keep = (ms != 0).astype(np.float32)
        keep = np.ascontiguousarray(
            keep.reshape(BL, 4, 128).transpose(2, 0, 1).reshape(128, BL * 4))
        mkv = (keep * np.float32(cv)).astype(np.float32)
        mk8 = _f8(np.broadcast_to(keep[:, :, None], (128, BL * 4, 128)))
        in_maps.append({
            "xt": xt, "x8": x8, "wq8": wq8, "wk8": wk8, "wv8": wv8,
            "wo8": wo8, "wso": wso_b, "wi8": wi8, "wib": wib,
            "wt8": wt8, "wtb": wtb, "mkv": mkv, "mk8": mk8,
            "ident": identb, "onesrow": onesr,
            "brow": brow, "bicol": bicol, "gb": gbt,
        })

    trace = os.environ.get("KERNEL_TRACE", "0") == "1"
    res = run_bass_kernel_spmd(nc, in_maps, core_ids=list(range(NCORES)),
                               trace=trace)
    if trace and res.exec_time_ns is not None:
        print(f"HW exec time: {res.exec_time_ns} ns")
        if res.instructions_and_trace is not None:
            print(f"trace: {res.instructions_and_trace[1]}")
    out = np.concatenate([r["out"].reshape(BL, S, E) for r in res.results],
                         axis=0)
    return np.ascontiguousarray(out.astype(np.float32))


# revision 32
# speedup vs baseline: 1.0139x; 1.0139x over previous
"""BERT-base encoder layer on 8 Trainium2 NeuronCores (Bass/Tile).

Sharding: data-parallel over batch. Full inputs [32, 512, 768] split into 8
shards of 4 batches (2048 tokens); every core runs the same NEFF on its shard
(SPMD, no collectives); host concatenates the outputs.

v3 design (vs v2 bf16 baseline): fp8 e4m3 DoubleRow matmuls on the PE
wherever the quantization error stays within the 2e-2 budget. A DoubleRow
matmul pairs two 128-row contraction slabs in one instruction (moving
[128,2,512] fp8 streamed 2 values/lane/cycle), halving PE time per
256-contraction vs bf16 (measured: 214ns per DR matmul vs 2x214 bf16).
- QKV and O-proj: pure fp8 DoubleRow (errors are diluted ~10x by the x
  residual before reaching the trunk).
- Attention A@V + softmax denominator: all fp8 DoubleRow. DR cannot target
  PSUM partition offset 64 (hw forces the full col-group), so the odd head
  of each pair uses a FULL-width 128-col stationary: rows 64-127 get its
  numerator/denominator at the correct lanes and rows 0-63 compute a
  garbage product that the even head's start=True 64-wide group then
  overwrites (the PE executes in order, so this is race-free).
- The key mask is multiplicative: folded into the V epilogue (per-partition
  cv*keep scalars) and into the denominator's fp8 stationary, so exp runs
  UNMASKED as one [128,1024] activation per head per key-chunk-pair
  (24/batch instead of 48 biased exps).
- FFN fractionally fp8: all 3 wi contraction pairs + 8 of 12 wout pairs in
  fp8 DoubleRow; the last 8 wout chunks stay bf16 (host pre-scales them by
  swout so the psum scale stays uniform). Measured rel err 1.83e-2 vs the
  2e-2 gate; a CPU quantization sim (sim_fp8.py) tracks hw to <1% relative.
- Trunk-critical Wso GEMM and QK^T score matmuls stay bf16 (Wso errors hit
  the residual stream 1:1; scores have only 64-deep contraction so fp8
  DoubleRow cannot apply anyway).
All fp8 descale constants fold into existing epilogue ops (copy consts, the
gelu input scale, the LN psum scale), so fp8 adds no vector/scalar work.
Scores are computed transposed (keys on partitions) exactly as in v2; exp
output IS the P^T pair-slab layout the A@V DoubleRow needs.

Superphase A = QKV + attention + O-proj (stores xa = x + attn out);
superphase B = SelfOutput GEMM + LN1 + FFN + LN2, software-pipelined
[wso(b+1), htrans(b), wout(b-1), wi(b)] so LN1 latency and the hT/ffT psum
drains hide under GEMM windows; wso(0)+LN1(0) runs in superphase A's tail
under the last exp window (with its LN writes on the scalar engine); the
final iteration runs wi(3) before wout(2) so wout(3) starts gelu-stall-free
(ffT pools sized for 2 batches to permit that reorder).
"""

import os
import numpy as np
import ml_dtypes

B, S, E, H, DK, FF = 32, 512, 768, 12, 64, 3072
NCORES = 8
BL = B // NCORES          # batches per core = 4
T = BL * S                # tokens per core = 2048
EPS = 1e-12

SX_DEF = 32.0             # x fp8 scale (absmax ~5.4 -> ~170 < 240)
SV = 8.0                  # v fp8 scale
SA = 8.0                  # attention-output fp8 scale
SH = 8.0                  # h fp8 scale

_CACHE = {}


def _bf(a):
    return np.ascontiguousarray(np.asarray(a, np.float32).astype(ml_dtypes.bfloat16))


def _f8(a):
    a = np.clip(np.asarray(a, np.float32), -240.0, 240.0)
    return np.ascontiguousarray(a.astype(ml_dtypes.float8_e4m3))


def _pack3(w):
    """(K, N) -> (K//256*128, 2, N): DoubleRow slab pairs along dim 1."""
    K, N = w.shape
    t = w.reshape(K // 256, 2, 128, N).transpose(0, 2, 1, 3)
    return np.ascontiguousarray(t.reshape(K // 256 * 128, 2, N))


def _build(key):
    import concourse.bass as bass
    import concourse.bacc as bacc
    import concourse.mybir as mybir
    import concourse.tile as tile
    from contextlib import ExitStack

    (flags, cq, ck, cv, cav, co, cg, cw, sh) = key
    (use_bq, use_bk, use_bv, use_bo, use_bso, use_bi, use_bout,
     use_g1, use_b1, use_g2, use_b2) = flags

    AF = mybir.ActivationFunctionType
    OP = mybir.AluOpType
    AX = mybir.AxisListType
    BF16 = mybir.dt.bfloat16
    F32 = mybir.dt.float32
    F8 = mybir.dt.float8e4
    DR = mybir.MatmulPerfMode.DoubleRow

    nc = bacc.Bacc("TRN2", target_bir_lowering=False)

    d_xt = nc.dram_tensor("xt", (E, T), BF16, kind="ExternalInput")
    d_x8 = nc.dram_tensor("x8", (E // 2, 2, T), F8, kind="ExternalInput")
    d_wq8 = nc.dram_tensor("wq8", (E // 2, 2, E), F8, kind="ExternalInput")
    d_wk8 = nc.dram_tensor("wk8", (E // 2, 2, E), F8, kind="ExternalInput")
    d_wv8 = nc.dram_tensor("wv8", (E // 2, 2, E), F8, kind="ExternalInput")
    d_wo8 = nc.dram_tensor("wo8", (E // 2, 2, E), F8, kind="ExternalInput")
    d_wso = nc.dram_tensor("wso", (E, E), BF16, kind="ExternalInput")
    # FFN weights: first WI_P8/WT_P8 contraction pairs ship as fp8
    # DoubleRow pairs; the remaining chunks ship bf16 (pre-scaled by the
    # same swi/swout so the psum scale stays uniform)
    d_wi8 = nc.dram_tensor("wi8", (384, 2, FF), F8, kind="ExternalInput")
    d_wib = nc.dram_tensor("wib", (1, FF), BF16, kind="ExternalInput")
    d_wt8 = nc.dram_tensor("wt8", (1024, 2, E), F8, kind="ExternalInput")
    d_wtb = nc.dram_tensor("wtb", (1024, E), BF16, kind="ExternalInput")
    # mask, as multiplicative keep columns: mkv[p, b*4+jc] = cv*keep (f32,
    # folded into the V epilogue) and mk8[p, b*4+jc, :] = keep broadcast 64
    # wide (fp8 stationary for the softmax-denominator matmuls)
    d_mkv = nc.dram_tensor("mkv", (128, BL * 4), F32, kind="ExternalInput")
    d_mk8 = nc.dram_tensor("mk8", (128, BL * 4, 128), F8, kind="ExternalInput")
    d_id = nc.dram_tensor("ident", (128, 128), BF16, kind="ExternalInput")
    d_ones = nc.dram_tensor("onesrow", (1, 512), BF16, kind="ExternalInput")
    # bias rows (host pre-scaled into fp8-psum units where applicable):
    # 0=bq/8*sx*swq, 1=bk*sx*swk, 2=bv*sx*swv, 3=bo*sa*swo, 4=bso,
    # 5=bout*swout, 6=bi
    d_brow = nc.dram_tensor("brow", (7, FF), BF16, kind="ExternalInput")
    d_bic = nc.dram_tensor("bicol", (128, FF // 128), F32, kind="ExternalInput")
    # gamma1 | gamma2 | beta1 | beta2 ([128, 768] each, partition-broadcast)
    # + a trailing always-zero [128, 512] scratch region
    d_gb = nc.dram_tensor("gb", (128, 4 * E + 512), F32, kind="ExternalInput")
    d_out = nc.dram_tensor("out", (T, E), F32, kind="ExternalOutput")

    KT_E = E // 128    # 6
    KP_E = E // 256    # 3 DoubleRow pairs over the embedding dim
    NT_B = S // 128    # 4
    FT = FF // 128     # 24
    FP = FF // 256     # 12 DoubleRow pairs over the FFN dim
    WI_P8 = 3          # wi contraction pairs in fp8 (of 3); rest bf16
    WT_P8 = 8          # wout contraction pairs in fp8 (of 12); rest bf16

    need_gb = use_g1 or use_b1 or use_g2 or use_b2
    need_brow = use_bq or use_bk or use_bv or use_bo or use_bso or use_bout

    with ExitStack() as ctx:
        tc = ctx.enter_context(tile.TileContext(nc))

        c_pool = ctx.enter_context(tc.tile_pool(name="consts", bufs=1))
        # Wso hoisted to outer scope so its DMA overlaps superphase A
        wso_pool = ctx.enter_context(tc.tile_pool(name="wso", bufs=KT_E))
        # xa = x + attention output (feature-major), crosses the A->B boundary
        xa_pool = ctx.enter_context(tc.tile_pool(name="xa", bufs=BL * KT_E))
        # LN machinery is shared between the phases: wso(0)+LN1(0) runs in
        # superphase A's tail (PE slack under the last exp window)
        h_pool = ctx.enter_context(tc.tile_pool(name="h", bufs=3 * NT_B))
        sq_pool = ctx.enter_context(tc.tile_pool(name="sq", bufs=2))
        rs_pool = ctx.enter_context(tc.tile_pool(name="rsd", bufs=2))
        t_pool = ctx.enter_context(tc.tile_pool(name="sb_s", bufs=12))

        XA = {}    # (b, kt) -> [128, S] bf16 tile

        def layernorm(chunks, h_dst, gcol, use_g, use_bb, resid=None,
                      pscale=1.0, scalar_writes=False):
            """chunks: [(psum_ap, col0, n)]; h_dst: [128, E] out.
            resid: parallel list of sbuf APs added to psum*pscale first.

            Stages the psum chunks into SBUF immediately (frees the PSUM
            banks for the next GEMM group); eps=1e-12 dropped (var >> eps).
            References ones/gb/zeros, which are loaded before first use."""
            rtile = rs_pool.tile([128, E], F32, name="rt", tag="rsd")
            s1 = t_pool.tile([128, 1], F32, name="s1", tag="s1")
            s1b = t_pool.tile([128, 1], F32, name="s1b", tag="s1b")
            for i, ((ps, c0, n), acc) in enumerate(zip(chunks, (s1, s1b))):
                rx = resid[i] if resid is not None else zeros[:, :n]
                nc.vector.scalar_tensor_tensor(
                    rtile[:, c0:c0 + n], ps, pscale, rx,
                    op0=OP.mult, op1=OP.add)
                nc.vector.reduce_sum(acc[:, :], rtile[:, c0:c0 + n],
                                     axis=AX.X)
            srcs = [(rtile[:, c0:c0 + n], c0, n) for (_, c0, n) in chunks]
            mu_n = t_pool.tile([128, 1], F32, name="mun", tag="mun")
            nc.vector.tensor_scalar(           # mu_n = -(s1 + s1b)/E
                mu_n[:, :], s1[:, :], s1b[:, :], -1.0 / E,
                op0=OP.add, op1=OP.mult)
            ss = t_pool.tile([128, 1], F32, name="ssa", tag="ssa", bufs=34)
            ssb = t_pool.tile([128, 1], F32, name="ssb", tag="ssb", bufs=34)
            for (src, c0, n), acc in zip(srcs, (ss, ssb)):
                sq = sq_pool.tile([128, 512], BF16, name="sqt", tag="sq")
                nc.scalar.activation(sq[:, :n], src, AF.Square,
                                     accum_out=acc[:, :])
            v1 = t_pool.tile([128, 1], F32, name="v1", tag="v1")
            nc.vector.tensor_scalar(           # (ss+ssb)/E
                v1[:, :], ss[:, :], ssb[:, :], 1.0 / E,
                op0=OP.add, op1=OP.mult)
            musq = t_pool.tile([128, 1], F32, name="musq", tag="musq")
            nc.vector.scalar_tensor_tensor(    # mu^2
                musq[:, :], mu_n[:, :], 1.0, mu_n[:, :],
                op0=OP.mult, op1=OP.mult)
            var = t_pool.tile([128, 1], F32, name="var", tag="var")
            nc.vector.scalar_tensor_tensor(    # var = v1 - mu^2
                var[:, :], musq[:, :], -1.0, v1[:, :],
                op0=OP.mult, op1=OP.add)
            sd = t_pool.tile([128, 1], F32, name="sd", tag="sd")
            nc.scalar.sqrt(sd[:, :], var[:, :])
            rstd = t_pool.tile([128, 1], F32, name="rstd", tag="rstd")
            nc.vector.reciprocal(rstd[:, :], sd[:, :])
            if scalar_writes:
                # (x + mu_n)*rstd == x*rstd + mu_n*rstd -> scalar ACT
                mnr = t_pool.tile([128, 1], F32, name="mnr", tag="mnr")
                nc.vector.scalar_tensor_tensor(
                    mnr[:, :], mu_n[:, :], 1.0, rstd[:, :],
                    op0=OP.mult, op1=OP.mult)
                for (src, c0, n) in srcs:
                    nc.scalar.activation(
                        h_dst[:, c0:c0 + n], src, AF.Identity,
                        bias=mnr[:, :], scale=rstd[:, :])
            else:
                for (src, c0, n) in srcs:      # (x - mu) * rstd
                    nc.vector.tensor_scalar(
                        h_dst[:, c0:c0 + n], src, mu_n[:, :], rstd[:, :],
                        op0=OP.add, op1=OP.mult)
            if use_g:
                nc.vector.scalar_tensor_tensor(
                    h_dst[:, :], h_dst[:, :], 1.0,
                    gb[:, gcol * E:(gcol + 1) * E],
                    op0=OP.mult, op1=OP.mult)
            if use_bb:
                nc.vector.scalar_tensor_tensor(
                    h_dst[:, :], h_dst[:, :], 1.0,
                    gb[:, (gcol + 2) * E:(gcol + 3) * E],
                    op0=OP.mult, op1=OP.add)

        def wso_tt(b, tt, pmm, scalar_writes=False):
            """One token-tile of the SelfOutput GEMM + LN1 -> h tile (bf16)."""
            ch = []
            for ec, n in ((0, 512), (512, 256)):
                ps = pmm.tile([128, 512], F32, name="sops", tag="mm")
                for k in range(KT_E):
                    nc.tensor.matmul(
                        ps[:, :n], XA[(b, k)][:, tt * 128:(tt + 1) * 128],
                        WSO[k][:, ec:ec + n],
                        start=(k == 0), stop=(k == KT_E - 1 and not use_bso))
                if use_bso:
                    nc.tensor.matmul(
                        ps[:, :n], ones[0:1, 0:128],
                        brow[4:5, ec:ec + n], start=False, stop=True)
                ch.append((ps[:, :n], ec, n))
            ht = h_pool.tile([128, E], BF16, name="hht", tag="h")
            layernorm(ch, ht, 0, use_g1, use_b1, scalar_writes=scalar_writes)
            return ht

        # ============== superphase A: QKV, attention, O-proj ==============
        with ExitStack() as sa:
            w8_pool = sa.enter_context(tc.tile_pool(name="w8", bufs=KP_E))
            xt_pool = sa.enter_context(tc.tile_pool(name="xt",
                                                    bufs=2 * KT_E))
            x8_pool = sa.enter_context(tc.tile_pool(name="x8",
                                                    bufs=2 * KP_E + 1))
            qt_pool = sa.enter_context(tc.tile_pool(name="qt", bufs=KT_E + 1))
            kt_pool = sa.enter_context(tc.tile_pool(name="kt", bufs=KT_E + 1))
            v_pool = sa.enter_context(tc.tile_pool(name="v", bufs=4))
            pexp_pool = sa.enter_context(tc.tile_pool(name="pexp", bufs=24))
            rsb_pool = sa.enter_context(tc.tile_pool(name="rsb", bufs=2))
            att_pool = sa.enter_context(tc.tile_pool(name="att", bufs=7))

            p_mm = sa.enter_context(tc.tile_pool(name="p_mm", bufs=2,
                                                 space="PSUM"))
            p_sc = sa.enter_context(tc.tile_pool(name="p_sc", bufs=2,
                                                 space="PSUM"))
            p_acc = sa.enter_context(tc.tile_pool(name="p_acc", bufs=2,
                                                  space="PSUM"))

            def load_x8(b, split=False):
                x8s = []
                engs = ((nc.gpsimd, nc.sync, nc.scalar) if split
                        else (nc.gpsimd, nc.gpsimd, nc.gpsimd))
                for kp in range(KP_E):
                    t = x8_pool.tile([128, 2, S], F8, name="x8t", tag="x8")
                    engs[kp].dma_start(
                        t[:, :, :],
                        d_x8[kp * 128:(kp + 1) * 128, :, b * S:(b + 1) * S])
                    x8s.append(t)
                return x8s

            def load_xt(b, split=False):
                xts = []
                for k in range(KT_E):
                    t = xt_pool.tile([128, S], BF16, name="xtt", tag="xt")
                    eng = nc.sync if (split and k >= 4) else nc.gpsimd
                    eng.dma_start(
                        t[:, :], d_xt[k * 128:(k + 1) * 128, b * S:(b + 1) * S])
                    xts.append(t)
                return xts

            def load_x(b):
                return load_xt(b), load_x8(b)

            # startup: the first qk matmul only needs WQ8[0]+X8[0]; lead
            # each DMA queue with one of those, everything else behind
            def _load8(dram, kp, eng, name, width=E):
                t = w8_pool.tile([128, 2, width], F8, name=name, tag=name)
                eng.dma_start(t[:, :, :], dram[kp * 128:(kp + 1) * 128, :, :])
                return t

            WQ8 = [_load8(d_wq8, kp, (nc.sync, nc.scalar, nc.gpsimd)[kp],
                          "wqt") for kp in range(KP_E)]
            X8_cur = load_x8(0, split=True)
            WK8 = [_load8(d_wk8, kp, (nc.scalar, nc.sync, nc.gpsimd)[kp],
                          "wkt") for kp in range(KP_E)]
            XT_cur = load_xt(0, split=True)
            mkv = c_pool.tile_from(d_mkv[:, :], name="mkv")
            mk8 = c_pool.tile([128, BL * 4, 128], F8, name="mk8")
            nc.scalar.dma_start(mk8[:, :, :], d_mk8[:, :, :])
            # prewarm the EXP act-table while the PE waits on weight DMAs,
            # taking the 1.3us table load off the first score's critical path
            warm = t_pool.tile([128, 1], F32, name="warm", tag="warm")
            nc.scalar.activation(warm[:, :], mkv[:, 0:1], AF.Exp)
            WV8 = [_load8(d_wv8, kp, (nc.sync if kp % 2 == 0
                                      else nc.gpsimd), "wvt")
                   for kp in range(KP_E)]
            WO8 = [_load8(d_wo8, kp, (nc.scalar if kp % 2 == 0
                                      else nc.sync), "wot")
                   for kp in range(KP_E)]
            WSO = [wso_pool.tile([128, E], BF16, name="wsot", tag="wsot")
                   for _ in range(KT_E)]
            for k in range(KT_E):
                nc.sync.dma_start(WSO[k][:, :],
                                  d_wso[k * 128:(k + 1) * 128, :])
            ident = c_pool.tile_from(d_id[:, :], name="ident")
            ones = c_pool.tile_from(d_ones[:, :], name="ones")
            # trailing columns of d_gb are always zero-filled by the host
            zeros = c_pool.tile_from(d_gb[:, 4 * E:4 * E + 512], name="zeros")
            brow = (c_pool.tile_from(d_brow[:, :], name="brow")
                    if need_brow else None)
            gb = c_pool.tile_from(d_gb[:, :], name="gb") if need_gb else None

            ATT_prev, XT_prev = None, None
            H0 = []    # h(0) tiles produced in this phase's tail

            for b in range(BL):
                XT, X8 = XT_cur, X8_cur
                if b + 1 < BL:
                    XT_next, X8_next = load_x(b + 1)

                QT, KTt = [None] * KT_E, [None] * KT_E
                V8 = [None] * 2
                PEXP = {}
                ATT8 = [None] * KP_E

                def qk(et):
                    for W8, dstl, pool, ub, brx, tg, cst in (
                            (WQ8, QT, qt_pool, use_bq, 0, "qt", cq),
                            (WK8, KTt, kt_pool, use_bk, 1, "kt", ck)):
                        ps = p_mm.tile([128, S], F32, name="qkps", tag="mm")
                        for kp in range(KP_E):
                            nc.tensor.matmul(
                                ps[:, :],
                                W8[kp][:, :, et * 128:(et + 1) * 128],
                                X8[kp][:, :, :], perf_mode=DR,
                                start=(kp == 0),
                                stop=(kp == KP_E - 1 and not ub))
                        if ub:
                            nc.tensor.matmul(
                                ps[:, :],
                                brow[brx:brx + 1, et * 128:(et + 1) * 128],
                                ones[0:1, 0:S], start=False, stop=True)
                        dstl[et] = pool.tile([128, S], BF16, name="qkt",
                                             tag=tg)
                        nc.vector.tensor_scalar_mul(dstl[et][:, :], ps[:, :],
                                                    cst)

                def sc(hp, it0):
                    # scores^T for head pair hp, key chunks it0, it0+1:
                    # psum[k-chunk, q] = (KT[h] chunk).T @ QT[h]; exp is
                    # UNMASKED (mask lives in the V rows + denominator)
                    p = it0 // 2
                    for hh in range(2):
                        o = hh * 64
                        ps = p_sc.tile([128, 2 * S], F32, name="scps",
                                       tag="sc")
                        for it in (it0, it0 + 1):
                            nc.tensor.matmul(
                                ps[:, (it - it0) * S:(it - it0 + 1) * S],
                                KTt[hp][o:o + 64, it * 128:(it + 1) * 128],
                                QT[hp][o:o + 64, :], start=True, stop=True)
                        hd = 2 * hp + hh
                        if PEXP.get((hd, p)) is None:
                            PEXP[(hd, p)] = pexp_pool.tile(
                                [128, 2, S], F8, name="pexp", tag="pe")
                        nc.scalar.activation(
                            PEXP[(hd, p)][:, :, :], ps[:, :], AF.Exp)

                def vproj_chunk(tt, ec, n):
                    if V8[tt // 2] is None:
                        V8[tt // 2] = v_pool.tile([128, 2, E], F8, name="vt",
                                                  tag="v")
                    ps = p_mm.tile([128, 512], F32, name="vps", tag="mm")
                    for kp in range(KP_E):
                        nc.tensor.matmul(
                            ps[:, :n],
                            X8[kp][:, :, tt * 128:(tt + 1) * 128],
                            WV8[kp][:, :, ec:ec + n], perf_mode=DR,
                            start=(kp == 0),
                            stop=(kp == KP_E - 1 and not use_bv))
                    if use_bv:
                        nc.tensor.matmul(
                            ps[:, :n], ones[0:1, 0:128],
                            brow[2:3, ec:ec + n], start=False, stop=True)
                    nc.vector.tensor_scalar_mul(
                        V8[tt // 2][:, tt % 2, ec:ec + n], ps[:, :n],
                        mkv[:, b * 4 + tt:b * 4 + tt + 1])

                def vproj(tt):
                    vproj_chunk(tt, 0, 512)
                    vproj_chunk(tt, 512, 256)

                def oproj(et):
                    ps = p_mm.tile([128, S], F32, name="ops", tag="mm")
                    for p in range(KP_E):
                        nc.tensor.matmul(
                            ps[:, :], WO8[p][:, :, et * 128:(et + 1) * 128],
                            ATT_prev[p][:, :, :], perf_mode=DR,
                            start=(p == 0),
                            stop=(p == KP_E - 1 and not use_bo))
                    if use_bo:
                        nc.tensor.matmul(
                            ps[:, :], brow[3:4, et * 128:(et + 1) * 128],
                            ones[0:1, 0:S], start=False, stop=True)
                    xat = xa_pool.tile([128, S], BF16, name="xat", tag="xa")
                    nc.vector.scalar_tensor_tensor(
                        xat[:, :], ps[:, :], co, XT_prev[et][:, :],
                        op0=OP.mult, op1=OP.add)
                    XA[(b - 1, et)] = xat

                def av(hp):
                    # DoubleRow cannot target psum partition 64 (col-group
                    # clash), but a FULL-width (128-col) stationary lands the
                    # odd head's numerator on rows 64-127 legally -- rows
                    # 0-63 compute a garbage product against V_even that the
                    # even head's start=True 64-wide group then overwrites
                    # (PE executes in order, so no race).
                    aps = p_acc.tile([128, S], F32, name="avps", tag="acc")
                    sps = p_acc.tile([128, S], F32, name="smps", tag="acc")
                    mks = mk8[:, b * 4:b * 4 + 4, :]
                    for p in range(2):   # odd head: rows 64-127 (+garbage)
                        nc.tensor.matmul(
                            aps[:, :],
                            V8[p][:, :, hp * 128:(hp + 1) * 128],
                            PEXP[(2 * hp + 1, p)][:, :, :],
                            perf_mode=DR, start=(p == 0), stop=(p == 1))
                        nc.tensor.matmul(
                            sps[:, :], mks[:, 2 * p:2 * p + 2, :],
                            PEXP[(2 * hp + 1, p)][:, :, :],
                            perf_mode=DR, start=(p == 0), stop=(p == 1))
                    for p in range(2):   # even head: resets rows 0-63
                        nc.tensor.matmul(
                            aps[0:64, :],
                            V8[p][:, :, hp * 128:hp * 128 + 64],
                            PEXP[(2 * hp, p)][:, :, :],
                            perf_mode=DR, start=(p == 0), stop=(p == 1),
                            tile_position=(0, 0), skip_group_check=True)
                        nc.tensor.matmul(
                            sps[0:64, :], mks[:, 2 * p:2 * p + 2, 0:64],
                            PEXP[(2 * hp, p)][:, :, :],
                            perf_mode=DR, start=(p == 0), stop=(p == 1),
                            tile_position=(0, 0), skip_group_check=True)
                    rsb = rsb_pool.tile([128, S], F32, name="rsb", tag="rsb")
                    # ~18-bit reciprocal, ~5x faster than nc.vector.reciprocal;
                    # sums are in [~1, 600] so no denorm/inf edge cases
                    nc.vector.reciprocal_approx_fast(rsb[:, :], sps[:, :])
                    if ATT8[hp // 2] is None:
                        ATT8[hp // 2] = att_pool.tile([128, 2, S], F8,
                                                      name="attt", tag="att")
                    nc.vector.scalar_tensor_tensor(
                        ATT8[hp // 2][:, hp % 2, :], aps[:, :], cav,
                        rsb[:, :], op0=OP.mult, op1=OP.mult)

                nop = lambda: None
                O = [(lambda et=et: oproj(et)) if b > 0 else nop
                     for et in range(KT_E)]
                # batch 3 also runs wso(0)+LN1(0) under its exp-window slack
                W0 = ([(lambda tt=tt: H0.append(wso_tt(0, tt, p_mm)))
                       for tt in range(NT_B)] if b == BL - 1 else [nop] * 4)
                # interleaved emission: exp window overlaps V/O-proj GEMMs
                sched = [
                    lambda: qk(0), lambda: qk(1),
                    lambda: sc(0, 0), lambda: qk(2),
                    lambda: sc(0, 2), lambda: qk(3),
                    lambda: sc(1, 0), lambda: qk(4),
                    lambda: sc(1, 2), lambda: qk(5),
                    lambda: sc(2, 0), lambda: vproj(0),
                    lambda: sc(2, 2), lambda: vproj(1),
                    lambda: sc(3, 0), lambda: vproj(2),
                    lambda: sc(3, 2), lambda: vproj(3),
                    lambda: sc(4, 0), lambda: av(0),
                    lambda: sc(4, 2), O[0],
                    lambda: sc(5, 0), O[1],
                    lambda: sc(5, 2), O[2],
                    lambda: av(1), O[3],
                    W0[0], lambda: av(2),
                    O[4], W0[1],
                    lambda: av(3), O[5],
                    W0[2], W0[3],
                    lambda: av(4), lambda: av(5),
                ]
                for seg in sched:
                    seg()

                ATT_prev = ATT8
                XT_prev = XT
                if b + 1 < BL:
                    XT_cur, X8_cur = XT_next, X8_next

            # O-projection for the last batch
            for et in range(KT_E):
                ps = p_mm.tile([128, S], F32, name="ops", tag="mm")
                for p in range(KP_E):
                    nc.tensor.matmul(
                        ps[:, :], WO8[p][:, :, et * 128:(et + 1) * 128],
                        ATT_prev[p][:, :, :], perf_mode=DR,
                        start=(p == 0), stop=(p == KP_E - 1 and not use_bo))
                if use_bo:
                    nc.tensor.matmul(
                        ps[:, :], brow[3:4, et * 128:(et + 1) * 128],
                        ones[0:1, 0:S], start=False, stop=True)
                xat = xa_pool.tile([128, S], BF16, name="xat", tag="xa")
                nc.vector.scalar_tensor_tensor(
                    xat[:, :], ps[:, :], co, XT_prev[et][:, :],
                    op0=OP.mult, op1=OP.add)
                XA[(BL - 1, et)] = xat

        # ========= superphase B: SelfOutput GEMM + LN1, FFN, LN2 =========
        with ExitStack() as sb:
            wi_pool = sb.enter_context(tc.tile_pool(name="wi", bufs=WI_P8))
            wib_pool = (sb.enter_context(tc.tile_pool(
                name="wib", bufs=KT_E - 2 * WI_P8))
                if KT_E > 2 * WI_P8 else None)
            wout_pool = sb.enter_context(tc.tile_pool(name="wout",
                                                      bufs=WT_P8))
            wtb_pool = sb.enter_context(tc.tile_pool(name="wtb",
                                                     bufs=FT - 2 * WT_P8))
            b_pool = sb.enter_context(tc.tile_pool(name="b_consts", bufs=1))
            ht_pool = sb.enter_context(tc.tile_pool(name="ht",
                                                    bufs=2 * WI_P8 + 1))
            htb_pool = (sb.enter_context(tc.tile_pool(
                name="htb", bufs=2 * (KT_E - 2 * WI_P8) + 1))
                if KT_E > 2 * WI_P8 else None)
            fft_pool = sb.enter_context(tc.tile_pool(name="fft",
                                                     bufs=2 * WT_P8 + 2))
            fftb_pool = sb.enter_context(tc.tile_pool(
                name="fftb", bufs=2 * (FT - 2 * WT_P8) + 2))
            out_pool = sb.enter_context(tc.tile_pool(name="outp", bufs=2))

            p_mm = sb.enter_context(tc.tile_pool(name="pb_mm", bufs=5,
                                                 space="PSUM"))
            p_tr = sb.enter_context(tc.tile_pool(name="pb_tr", bufs=3,
                                                 space="PSUM"))

            ENGS = (nc.sync, nc.gpsimd, nc.scalar)

            def _loadb(pool, dram, kp, name, qi):
                w = dram.shape[2]
                t = pool.tile([128, 2, w], F8, name=name, tag=name)
                ENGS[qi % 3].dma_start(t[:, :, :],
                                       dram[kp * 128:(kp + 1) * 128, :, :])
                return t

            WI8 = [_loadb(wi_pool, d_wi8, kp, "wi8t", kp)
                   for kp in range(WI_P8)]
            WIB = [wib_pool.tile([128, FF], BF16, name="wibt", tag="wibt")
                   for _ in range(KT_E - 2 * WI_P8)]
            for c in range(KT_E - 2 * WI_P8):
                ENGS[(WI_P8 + c) % 3].dma_start(
                    WIB[c][:, :], d_wib[c * 128:(c + 1) * 128, :])
            WT8 = [_loadb(wout_pool, d_wt8, fp, "wt8t", fp)
                   for fp in range(WT_P8)]
            WTB = [wtb_pool.tile([128, E], BF16, name="wtbt", tag="wtbt")
                   for _ in range(FT - 2 * WT_P8)]
            for c in range(FT - 2 * WT_P8):
                ENGS[(WT_P8 + c) % 3].dma_start(
                    WTB[c][:, :], d_wtb[c * 128:(c + 1) * 128, :])
            bic = b_pool.tile_from(d_bic[:, :], name="bic") if use_bi else None

            def emit_htrans(hh_t, sc_copies=False):
                hT = [ht_pool.tile([128, 2, S], F8, name="htt", tag="ht")
                      for _ in range(WI_P8)]
                hTb = [htb_pool.tile([128, S], BF16, name="htbt", tag="htb")
                       for _ in range(KT_E - 2 * WI_P8)]
                for tt in range(NT_B):
                    tps = [p_tr.tile([128, 512], BF16, name="htp", tag="tr")
                           for _ in range(2)]
                    for et in range(KT_E):
                        sl = tps[et // 4][:, (et % 4) * 128:(et % 4 + 1) * 128]
                        nc.tensor.transpose(
                            sl, hh_t[tt][:, et * 128:(et + 1) * 128],
                            ident[:, :])
                    for et in range(KT_E):
                        sl = tps[et // 4][:, (et % 4) * 128:(et % 4 + 1) * 128]
                        if et < 2 * WI_P8:
                            dst = hT[et // 2][:, et % 2,
                                              tt * 128:(tt + 1) * 128]
                        else:
                            dst = hTb[et - 2 * WI_P8][:,
                                                      tt * 128:(tt + 1) * 128]
                        if sc_copies:
                            nc.scalar.activation(dst, sl, AF.Copy,
                                                 scale=float(sh))
                        else:
                            nc.vector.tensor_scalar_mul(dst, sl, sh)
                return hT, hTb

            def emit_wi(hTp):
                hT, hTb = hTp
                nb = KT_E - 2 * WI_P8
                ffT = [None] * FP
                ffTb = [None] * (FT - 2 * WT_P8)
                for ft in range(FT):
                    ps = p_mm.tile([128, 512], F32, name="fips", tag="mm")
                    for kp in range(WI_P8):
                        nc.tensor.matmul(
                            ps[:, :], WI8[kp][:, :, ft * 128:(ft + 1) * 128],
                            hT[kp][:, :, :], perf_mode=DR,
                            start=(kp == 0),
                            stop=(kp == WI_P8 - 1 and nb == 0))
                    for c in range(nb):
                        nc.tensor.matmul(
                            ps[:, :], WIB[c][:, ft * 128:(ft + 1) * 128],
                            hTb[c][:, :],
                            start=False, stop=(c == nb - 1))
                    if ft < 2 * WT_P8:
                        if ffT[ft // 2] is None:
                            ffT[ft // 2] = fft_pool.tile(
                                [128, 2, 512], F8, name="fftt", tag="fft")
                        dst = ffT[ft // 2][:, ft % 2, :]
                    else:
                        ffTb[ft - 2 * WT_P8] = fftb_pool.tile(
                            [128, 512], BF16, name="fftbt", tag="fftb")
                        dst = ffTb[ft - 2 * WT_P8][:, :]
                    if use_bi:
                        nc.scalar.activation(dst, ps[:, :], AF.Gelu,
                                             bias=bic[:, ft:ft + 1], scale=cg)
                    else:
                        nc.scalar.activation(dst, ps[:, :], AF.Gelu, scale=cg)
                return ffT, ffTb

            def emit_wout(b, ffp, hh_t):
                ffT, ffTb = ffp
                t0 = b * S
                for tt in range(NT_B):
                    ch = []
                    nb = FT - 2 * WT_P8
                    for ec, n in ((0, 512), (512, 256)):
                        ps = p_mm.tile([128, 512], F32, name="wops", tag="mm")
                        for fp in range(WT_P8):
                            nc.tensor.matmul(
                                ps[:, :n],
                                ffT[fp][:, :, tt * 128:(tt + 1) * 128],
                                WT8[fp][:, :, ec:ec + n], perf_mode=DR,
                                start=(fp == 0), stop=False)
                        for c in range(nb):
                            nc.tensor.matmul(
                                ps[:, :n],
                                ffTb[c][:, tt * 128:(tt + 1) * 128],
                                WTB[c][:, ec:ec + n],
                                start=False,
                                stop=(c == nb - 1 and not use_bout))
                        if use_bout:
                            nc.tensor.matmul(
                                ps[:, :n], ones[0:1, 0:128],
                                brow[5:6, ec:ec + n], start=False, stop=True)
                        ch.append((ps[:, :n], ec, n))
                    otile = out_pool.tile([128, E], F32, name="ot", tag="outp")
                    resid = [hh_t[tt][:, ec:ec + n] for (_, ec, n) in ch]
                    layernorm(ch, otile, 1, use_g2, use_b2, resid=resid,
                              pscale=cw)
                    # sync queue keeps the slow gpsimd drain off the tail
                    nc.sync.dma_start(
                        d_out[t0 + tt * 128:t0 + (tt + 1) * 128, :],
                        otile[:, :])

            # software pipeline: h(0) computed in superphase A's tail (H0).
            # Per iteration: ht(b), wso(b+1), wout(b-1), wi(b) -- the LN1 and
            # LN2 sqrt activations cluster together between gelu runs (the
            # SQRT and GELU act-tables evict each other; interleaving them
            # per-token-tile cost ~19 table reloads of 1.3us each), and
            # wout(b-1) starts a full iteration after its gelus finished.
            h_ = [None] * BL
            hT_ = [None] * BL
            ff_ = [None] * BL
            h_[0] = H0
            h_[1] = [wso_tt(1, tt, p_mm, scalar_writes=True)
                     for tt in range(NT_B)]
            hT_[0] = emit_htrans(h_[0], sc_copies=True)
            ff_[0] = emit_wi(hT_[0])
            h_[2] = [wso_tt(2, tt, p_mm) for tt in range(NT_B)]
            hT_[1] = emit_htrans(h_[1], sc_copies=True)
            emit_wout(0, ff_[0], h_[0])
            ff_[1] = emit_wi(hT_[1])
            h_[3] = [wso_tt(3, tt, p_mm) for tt in range(NT_B)]
            hT_[2] = emit_htrans(h_[2])
            emit_wout(1, ff_[1], h_[1])
            ff_[2] = emit_wi(hT_[2])
            # last iteration: wi(3) before wout(2) so its gelus finish under
            # wout(2)'s matmul window and wout(3) starts without a stall
            hT_[3] = emit_htrans(h_[3])
            ff_[3] = emit_wi(hT_[3])
            emit_wout(2, ff_[2], h_[2])
            emit_wout(3, ff_[3], h_[3])

    nc.compile()
    return nc


def _get_program(key):
    if key not in _CACHE:
        _CACHE[key] = _build(key)
    return _CACHE[key]


def kernel(x, mask, Wq, bq, Wk, bk, Wv, bv, Wo, bo,
           Wso, bso, gso, beso, Wi, bi, Wout, bout, gout, beout):
    from concourse.bass_utils import run_bass_kernel_spmd

    x = np.asarray(x, np.float32)
    mask = np.asarray(mask)
    sc = 1.0 / float(np.sqrt(np.float32(DK)))

    wq_t = np.asarray(Wq, np.float32) * sc
    amax = lambda a: max(float(np.abs(np.asarray(a, np.float32)).max()), 1e-30)
    sx = min(SX_DEF, 224.0 / amax(x))
    swq, swk = 224.0 / amax(wq_t), 224.0 / amax(Wk)
    swv, swo = 224.0 / amax(Wv), 224.0 / amax(Wo)
    swi, swout = 224.0 / amax(Wi), 224.0 / amax(Wout)

    cq = 1.0 / (sx * swq)
    ck = 1.0 / (sx * swk)
    cv = SV / (sx * swv)
    cav = SA / SV
    co = 1.0 / (SA * swo)
    cg = 1.0 / (SH * swi)
    cw = 1.0 / swout

    z = lambda a: not np.any(np.asarray(a))
    one = lambda a: bool(np.all(np.asarray(a) == 1.0))
    flags = (not z(bq), not z(bk), not z(bv), not z(bo), not z(bso),
             not z(bi), not z(bout),
             not one(gso), not z(beso), not one(gout), not z(beout))
    key = (flags, cq, ck, cv, cav, co, cg, cw, SH)
    nc = _get_program(key)

    wq8 = _pack3(_f8(wq_t * swq))
    wk8 = _pack3(_f8(np.asarray(Wk, np.float32) * swk))
    wv8 = _pack3(_f8(np.asarray(Wv, np.float32) * swv))
    wo8 = _pack3(_f8(np.asarray(Wo, np.float32) * swo))
    wi_s = np.asarray(Wi, np.float32) * swi
    wt_s = np.asarray(Wout, np.float32) * swout
    wi8 = _pack3(_f8(wi_s))
    wib = _bf(np.zeros((1, FF), np.float32))
    wt8 = _pack3(_f8(wt_s[:2048]))
    wtb = _bf(wt_s[2048:])
    wso_b = _bf(Wso)
    identb = _bf(np.eye(128))
    onesr = _bf(np.ones((1, 512)))

    brow = np.zeros((7, FF), np.float32)
    brow[0, :E] = np.asarray(bq, np.float32) * sc * (sx * swq)
    brow[1, :E] = np.asarray(bk, np.float32) * (sx * swk)
    brow[2, :E] = np.asarray(bv, np.float32) * (sx * swv)
    brow[3, :E] = np.asarray(bo, np.float32) * (SA * swo)
    brow[4, :E] = bso
    brow[5, :E] = np.asarray(bout, np.float32) * swout
    brow[6, :] = bi
    brow = _bf(brow)
    bicol = np.asarray(bi, np.float32).reshape(FF // 128, 128).T.copy()
    gbt = np.zeros((128, 4 * E + 512), np.float32)
    for i, g in enumerate((gso, gout, beso, beout)):   # g1|g2|b1|b2
        gbt[:, i * E:(i + 1) * E] = np.broadcast_to(
            np.asarray(g, np.float32).reshape(1, E), (128, E))

    in_maps = []
    for c in range(NCORES):
        xs = x[c * BL:(c + 1) * BL].reshape(T, E)
        xsT = np.ascontiguousarray(xs.T)
        xt = _bf(xsT)
        x8 = _f8(xsT * sx).reshape(E // 256, 2, 128, T)
        x8 = np.ascontiguousarray(
            x8.transpose(0, 2, 1, 3).reshape(E // 2, 2, T))
        ms = np.asarray(mask[c * BL:(c + 1) * BL]).reshape(BL, S)
        # keep[p, b*4 + jc] = 0/1 for key token jc*128+p of batch b
        keep = (ms != 0).astype(np.float32)
        keep = np.ascontiguousarray(
            keep.reshape(BL, 4, 128).transpose(2, 0, 1).reshape(128, BL * 4))
        mkv = (keep * np.float32(cv)).astype(np.float32)
        mk8 = _f8(np.broadcast_to(keep[:, :, None], (128, BL * 4, 128)))
        in_maps.append({
            "xt": xt, "x8": x8, "wq8": wq8, "wk8": wk8, "wv8": wv8,
            "wo8": wo8, "wso": wso_b, "wi8": wi8, "wib": wib,
            "wt8": wt8, "wtb": wtb, "mkv": mkv, "mk8": mk8,
            "ident": identb, "onesrow": onesr,
            "brow": brow, "bicol": bicol, "gb": gbt,
        })

    trace = os.environ.get("KERNEL_TRACE", "0") == "1"
    res = run_bass_kernel_spmd(nc, in_maps, core_ids=list(range(NCORES)),
                               trace=trace)
    if trace and res.exec_time_ns is not None:
        print(f"HW exec time: {res.exec_time_ns} ns")
        if res.instructions_and_trace is not None:
            print(f"trace: {res.instructions_and_trace[1]}")
    out = np.concatenate([r["out"].reshape(BL, S, E) for r in res.results],
                         axis=0)
    return np.ascontiguousarray(out.astype(np.float32))
